# revision 1
# baseline (speedup 1.0000x reference)
"""Trainium2 Bass kernel for nn_EventSampler (thinning / rejection sampling).

Contract: kernel(**inputs) takes the FULL unsharded inputs (as produced by
setup_inputs()) and returns the full output (res, weights), matching the
jax reference. Internally shards the batch dim (16) across 8 NeuronCores
(2 batches = 256 (b,l) pairs per core) and runs a single SPMD Bass program.

Per (b,l) pair (one SBUF partition per pair, 128 pairs per chunk, 2 chunks
per core):
  unified grid: ONE [26, M] softplus-sum evaluation per pair covering the 20
    bound-scan points dt_s = tds*s/19 AND 6 Chebyshev-Lobatto nodes on
    [0, D] (D >= xmax picked on host from a float64 bound estimate; only the
    interpolation domain, never the math, depends on it).
  bounds = 1.5 * max over the 20 scan values.
  tot(x) at the 256 scaled draws x_e = raw_e/bounds is evaluated as the
    degree-5 Chebyshev interpolant (Clenshaw); interpolation error sits at
    the f32 noise floor because tot is analytic on this tiny domain.
  accept[s,e] = unif[s,e]*bounds < tot_e ; accepted time = min accepted x_e,
    computed as bounds-free max of accept/raw_e then one reciprocal and a
    final *1/bounds; fallback = max(x_last, dtime_boundary).

softplus = Ln(exp(z)+1) on ScalarE (Exp and Ln share one ACT table set).
Stage-2's [S,E] elementwise ops are split between VectorE and GpSimd by
s-range; the VectorE instances read their per-e operand from PSUM so the two
engines never touch the shared SBUF port pair at the same time.
"""

import os
import sys

import numpy as np

for _p in ("/opt/trn_rl_repo",):
    if _p not in sys.path and os.path.isdir(_p):
        sys.path.insert(0, _p)

import concourse.bacc as bacc
import concourse.tile as tile
import concourse.mybir as mybir
from concourse.bass_utils import run_bass_kernel_spmd

F32 = mybir.dt.float32

# Problem constants (hardcoded per the harness contract).
B, L, M = 16, 128, 32
S, E, S0 = 32, 256, 20          # NUM_SAMPLE, NUM_EXP, NUM_BOUND
OVER = 1.5
KC = 5                          # Chebyshev-Lobatto nodes for tot(x)
G = S0 + KC                     # unified grid points per pair
GS = 24                         # s-rows of stage-2 handled by GpSimd
N_CORES = 8
BPC = B // N_CORES              # batches per core
P = BPC * L                     # (b,l) pairs per core = 256
NP = 128                        # SBUF partitions
NCHUNK = P // NP                # chunks per core = 2

_CACHE = {}


def _alu(name):
    return getattr(mybir.AluOpType, name)


def _act(name):
    return getattr(mybir.ActivationFunctionType, name)


def build_program(gs=GS):
    nc = bacc.Bacc("TRN2", target_bir_lowering=False, debug=False,
                   enable_asserts=False, num_devices=N_CORES)

    # Per-core DRAM I/O.
    u_d = nc.dram_tensor("u", [P, S, E], F32, kind="ExternalInput")
    raw_d = nc.dram_tensor("raw", [P, E], F32, kind="ExternalInput")
    tds_d = nc.dram_tensor("tds", [P, 1], F32, kind="ExternalInput")
    dtb_d = nc.dram_tensor("dtb", [P, 1], F32, kind="ExternalInput")
    aemb_d = nc.dram_tensor("aemb", [P, M], F32, kind="ExternalInput")
    nodes_d = nc.dram_tensor("nodes", [P, KC], F32, kind="ExternalInput")
    fourd_d = nc.dram_tensor("fourd", [P, 1], F32, kind="ExternalInput")
    # Host-replicated small constants ([NP, ...]).
    negbeta_d = nc.dram_tensor("negbeta", [NP, M], F32, kind="ExternalInput")
    mu_d = nc.dram_tensor("mu", [NP, M], F32, kind="ExternalInput")
    tlin_d = nc.dram_tensor("tlin", [NP, S0], F32, kind="ExternalInput")
    wfull_d = nc.dram_tensor("wfull", [NP, KC * KC], F32, kind="ExternalInput")
    res_d = nc.dram_tensor("res", [P, S], F32, kind="ExternalOutput")

    mult = _alu("mult")
    add = _alu("add")
    sub = _alu("subtract")
    is_lt = _alu("is_lt")
    is_gt = _alu("is_gt")
    amax = _alu("max")
    amin = _alu("min")
    Exp = _act("Exp")
    Cp = _act("Copy")
    Ln = _act("Ln")
    DS = S - gs                   # s-rows on DVE

    with tile.TileContext(nc) as tc:
        with (
            tc.tile_pool(name="const", bufs=1) as constp,
            tc.tile_pool(name="cps", bufs=1, space="PSUM") as cps,
            tc.tile_pool(name="pps", bufs=2, space="PSUM") as pps,
            tc.tile_pool(name="cbp", bufs=1, space="PSUM") as cbp,
            tc.tile_pool(name="ubuf", bufs=2) as ubuf,
            tc.tile_pool(name="slab", bufs=1) as slab,
            tc.tile_pool(name="small", bufs=2) as small,
            tc.tile_pool(name="clen", bufs=1) as clen,
        ):
            negbeta_t = constp.tile([NP, M], F32, tag="negbeta")
            nc.sync.dma_start(out=negbeta_t[:], in_=negbeta_d.ap())
            mu_t = constp.tile([NP, M], F32, tag="mu")
            nc.sync.dma_start(out=mu_t[:], in_=mu_d.ap())
            tlin_t = constp.tile([NP, S0], F32, tag="tlin")
            nc.sync.dma_start(out=tlin_t[:], in_=tlin_d.ap())
            wfull_t = constp.tile([NP, KC * KC], F32, tag="wfull")
            nc.sync.dma_start(out=wfull_t[:], in_=wfull_d.ap())
            # PSUM copies of per-m constants (second operands of DVE tt ops)
            consts_p = cps.tile([NP, 2 * M + KC * KC], F32, tag="consts_p")
            nc.vector.tensor_copy(consts_p[:, 0:M], negbeta_t[:])
            nc.vector.tensor_copy(consts_p[:, M:2 * M], mu_t[:])
            nc.vector.tensor_copy(consts_p[:, 2 * M:], wfull_t[:])
            nb_e = consts_p[:, 0:M].unsqueeze(1)           # [NP,1,M] PSUM
            mu_e = consts_p[:, M:2 * M].unsqueeze(1)       # [NP,1,M] PSUM
            wfull_p = consts_p[:, 2 * M:].rearrange("p (a b) -> p a b", a=KC)

            # ---- phase 0: all small DMAs for both chunks (issued before
            # the big u loads so both stage-0/1 chains can start immediately),
            # then the u slabs. ----
            ch = [dict() for _ in range(NCHUNK)]
            for c in range(NCHUNK):
                sl = slice(c * NP, (c + 1) * NP)
                d = ch[c]
                d["raw_t"] = small.tile([NP, E], F32, tag="raw", name=f"raw{c}")
                nc.sync.dma_start(out=d["raw_t"][:], in_=raw_d.ap()[sl])
                d["tds_t"] = small.tile([NP, 1], F32, tag="tds", name=f"tds{c}")
                nc.sync.dma_start(out=d["tds_t"][:], in_=tds_d.ap()[sl])
                d["dtb_t"] = small.tile([NP, 1], F32, tag="dtb", name=f"dtb{c}")
                nc.sync.dma_start(out=d["dtb_t"][:], in_=dtb_d.ap()[sl])
                d["fourd_t"] = small.tile([NP, 1], F32, tag="fourd", name=f"fourd{c}")
                nc.sync.dma_start(out=d["fourd_t"][:], in_=fourd_d.ap()[sl])
                d["aemb_t"] = small.tile([NP, M], F32, tag="aemb", name=f"aemb{c}")
                nc.sync.dma_start(out=d["aemb_t"][:], in_=aemb_d.ap()[sl])
                d["pts"] = small.tile([NP, G], F32, tag="pts", name=f"pts{c}")
                nc.sync.dma_start(out=d["pts"][:, S0:G], in_=nodes_d.ap()[sl])
            for c in range(NCHUNK):
                sl = slice(c * NP, (c + 1) * NP)
                ch[c]["u_t"] = ubuf.tile([NP, S, E], F32, tag="u", name=f"u{c}")
                nc.sync.dma_start(out=ch[c]["u_t"][:], in_=u_d.ap()[sl])

            # ---- phase 1: bounds + Chebyshev tot for both chunks ----
            for c in range(NCHUNK):
                d = ch[c]
                raw_t, tds_t, aemb_t, pts = d["raw_t"], d["tds_t"], d["aemb_t"], d["pts"]
                aemb_e = aemb_t[:].unsqueeze(1)
                nc.scalar.activation(pts[:, 0:S0], tlin_t[:], Cp, scale=tds_t[:])
                zG = small.tile([NP, G, M], F32, tag="gA")
                nc.vector.tensor_tensor(out=zG[:], in0=pts[:].unsqueeze(2).to_broadcast((NP, G, M)),
                                        in1=nb_e.to_broadcast((NP, G, M)), op=mult)
                dG = small.tile([NP, G, M], F32, tag="gB")
                nc.scalar.activation(dG[:], zG[:], Exp)
                gG = small.tile([NP, G, M], F32, tag="gA")
                nc.vector.tensor_tensor(out=gG[:], in0=dG[:],
                                        in1=aemb_e.to_broadcast((NP, G, M)), op=mult)
                sG = small.tile([NP, G, M], F32, tag="gB")
                nc.vector.tensor_tensor(out=sG[:], in0=gG[:],
                                        in1=mu_e.to_broadcast((NP, G, M)), op=add)
                eG = small.tile([NP, G, M], F32, tag="gA")
                nc.scalar.activation(eG[:], sG[:], Exp)
                spG = small.tile([NP, G, M], F32, tag="gB")
                nc.scalar.activation(spG[:], eG[:], Ln, bias=1.0)
                vals = small.tile([NP, G], F32, tag="vals")
                nc.vector.reduce_sum(out=vals[:], in_=spG[:], axis=mybir.AxisListType.X)

                bmax = small.tile([NP, 1], F32, tag="bmax")
                nc.vector.reduce_max(out=bmax[:], in_=vals[:, 0:S0],
                                     axis=mybir.AxisListType.X)
                b15 = small.tile([NP, 1], F32, tag="b15")
                nc.scalar.activation(b15[:], bmax[:], Cp, scale=float(OVER))
                invb = small.tile([NP, 1], F32, tag="invb")
                nc.vector.reciprocal(invb[:], b15[:])
                svc2 = small.tile([NP, 1], F32, tag="svc2")
                nc.scalar.activation(svc2[:], invb[:], Cp, scale=d["fourd_t"][:])
                w2 = small.tile([NP, E], F32, tag="w2")
                nc.scalar.activation(w2[:], raw_t[:], Cp, scale=svc2[:], bias=-2.0)
                v = small.tile([NP, E], F32, tag="v")
                nc.scalar.activation(v[:], w2[:], Cp, scale=0.5)
                rawrec = small.tile([NP, E], F32, tag="rawrec")
                nc.vector.reciprocal(rawrec[:], raw_t[:])
                pchunk = pps.tile([NP, 2 * E], F32, tag="pchunk")
                rawrec_p = pchunk[:, E:2 * E]
                nc.scalar.activation(rawrec_p, rawrec[:], Cp)

                cw = small.tile([NP, KC, KC], F32, tag="cw")
                nc.vector.tensor_tensor(out=cw[:], in0=vals[:, S0:G].unsqueeze(1).to_broadcast((NP, KC, KC)),
                                        in1=wfull_p, op=mult)
                cc = small.tile([NP, KC], F32, tag="cc")
                nc.vector.reduce_sum(out=cc[:], in_=cw[:], axis=mybir.AxisListType.X)

                b1 = cbp.tile([NP, E], F32, tag="cbi")
                nc.vector.tensor_scalar(out=b1[:], in0=w2[:], scalar1=cc[:, KC - 1:KC],
                                        scalar2=cc[:, KC - 2:KC - 1], op0=mult, op1=add)
                b2ap = cc[:, KC - 1:KC].to_broadcast((NP, E))
                rot = ["cbA", "cbB", "cbi"]
                for i, k in enumerate(range(KC - 3, 0, -1)):
                    t_ = clen.tile([NP, E], F32, tag=f"cbt{k}")
                    nc.vector.tensor_tensor(out=t_[:], in0=w2[:], in1=b1[:], op=mult)
                    bn = cbp.tile([NP, E], F32, tag=rot[i % 3])
                    nc.vector.scalar_tensor_tensor(out=bn[:], in0=t_[:],
                                                   scalar=cc[:, k:k + 1], in1=b2ap,
                                                   op0=add, op1=sub)
                    b2ap = b1[:]
                    b1 = bn
                t_ = clen.tile([NP, E], F32, tag="cbt0")
                nc.vector.tensor_tensor(out=t_[:], in0=v[:], in1=b1[:], op=mult)
                tot = small.tile([NP, E], F32, tag="tot")
                nc.vector.scalar_tensor_tensor(out=tot[:], in0=t_[:],
                                               scalar=cc[:, 0:1], in1=b2ap,
                                               op0=add, op1=sub)
                tot_p = pchunk[:, 0:E]
                nc.scalar.activation(tot_p, tot[:], Cp)
                d.update(b15=b15, invb=invb, rawrec=rawrec, tot=tot,
                         pchunk=pchunk)

            # ---- phase 2: accept/reject + tail for both chunks ----
            for c in range(NCHUNK):
                sl = slice(c * NP, (c + 1) * NP)
                d = ch[c]
                u_t, b15, invb = d["u_t"], d["b15"], d["invb"]
                rawrec, tot, pchunk = d["rawrec"], d["tot"], d["pchunk"]
                tot_p = pchunk[:, 0:E]
                rawrec_p = pchunk[:, E:2 * E]
                rr_bd = rawrec_p.unsqueeze(1).to_broadcast((NP, DS, E))
                rr_bg = rawrec[:].unsqueeze(1).to_broadcast((NP, gs, E))

                h1 = gs // 2
                h2 = gs - h1
                # accept mask in three SEPARATE tiles (distinct tiles per
                # writer/reader pair -- slice-sharing raced on real HW) so
                # GpSimd starts multiplying after only h1 rows are compared.
                acc_g1 = slab.tile([NP, h1, E], F32, tag="accg1")
                nc.vector.scalar_tensor_tensor(out=acc_g1[:], in0=u_t[:, 0:h1, :],
                                               scalar=b15[:],
                                               in1=tot_p.unsqueeze(1).to_broadcast((NP, h1, E)),
                                               op0=mult, op1=is_lt)
                acc_g2 = slab.tile([NP, h2, E], F32, tag="accg2")
                nc.vector.scalar_tensor_tensor(out=acc_g2[:], in0=u_t[:, h1:gs, :],
                                               scalar=b15[:],
                                               in1=tot_p.unsqueeze(1).to_broadcast((NP, h2, E)),
                                               op0=mult, op1=is_lt)
                if DS > 0:
                    acc_d = slab.tile([NP, DS, E], F32, tag="accd")
                    nc.vector.scalar_tensor_tensor(out=acc_d[:], in0=u_t[:, gs:S, :],
                                                   scalar=b15[:],
                                                   in1=tot_p.unsqueeze(1).to_broadcast((NP, DS, E)),
                                                   op0=mult, op1=is_lt)
                sel_g1 = slab.tile([NP, h1, E], F32, tag="selg1")
                nc.gpsimd.tensor_tensor(out=sel_g1[:], in0=acc_g1[:],
                                        in1=rawrec[:].unsqueeze(1).to_broadcast((NP, h1, E)),
                                        op=mult)
                sel_g2 = slab.tile([NP, h2, E], F32, tag="selg2")
                nc.gpsimd.tensor_tensor(out=sel_g2[:], in0=acc_g2[:],
                                        in1=rawrec[:].unsqueeze(1).to_broadcast((NP, h2, E)),
                                        op=mult)
                if DS > 0:
                    sel_d = slab.tile([NP, DS, E], F32, tag="seld")
                    nc.vector.tensor_tensor(out=sel_d[:], in0=acc_d[:],
                                            in1=rr_bd, op=mult)
                    red_d = small.tile([NP, DS], F32, tag="redd")
                    nc.vector.reduce_max(out=red_d[:], in_=sel_d[:], axis=mybir.AxisListType.X)
                red_g1 = small.tile([NP, h1], F32, tag="redg1")
                nc.vector.reduce_max(out=red_g1[:], in_=sel_g1[:], axis=mybir.AxisListType.X)
                red_g2 = small.tile([NP, h2], F32, tag="redg2")
                nc.vector.reduce_max(out=red_g2[:], in_=sel_g2[:], axis=mybir.AxisListType.X)

                red = small.tile([NP, S], F32, tag="red")
                nc.scalar.activation(red[:, 0:h1], red_g1[:], Cp)
                nc.scalar.activation(red[:, h1:gs], red_g2[:], Cp)
                if DS > 0:
                    nc.scalar.activation(red[:, gs:S], red_d[:], Cp)

                accm = small.tile([NP, S], F32, tag="accm")
                nc.vector.reciprocal(accm[:], red[:])
                acc = small.tile([NP, S], F32, tag="acc")
                nc.scalar.activation(acc[:], accm[:], Cp, scale=invb[:])
                who = small.tile([NP, S], mybir.dt.int32, tag="who")
                nc.vector.tensor_scalar(out=who[:], in0=red[:], scalar1=0.0,
                                        scalar2=None, op0=is_gt)
                lastx = small.tile([NP, 1], F32, tag="lastx")
                nc.scalar.activation(lastx[:], d["raw_t"][:, E - 1:E], Cp, scale=invb[:])
                fb = small.tile([NP, 1], F32, tag="fb")
                nc.vector.tensor_tensor(out=fb[:], in0=lastx[:], in1=d["dtb_t"][:],
                                        op=amax)
                res_t = small.tile([NP, S], F32, tag="res")
                nc.scalar.activation(res_t[:], fb[:].to_broadcast((NP, S)), Cp)
                nc.vector.copy_predicated(res_t[:], who[:], acc[:])
                res2_t = small.tile([NP, S], F32, tag="res2")
                nc.vector.tensor_scalar(out=res2_t[:], in0=res_t[:], scalar1=1e5,
                                        scalar2=None, op0=amin)
                nc.sync.dma_start(out=res_d.ap()[sl], in_=res2_t[:])

    nc.finalize()
    return nc


def _prep_inputs(time_seq, time_delta_seq, event_seq, dtime_boundary, exp_raw,
                 unif_numbers, mu, alpha, beta, type_emb):
    f = np.float32
    tds = np.ascontiguousarray(np.asarray(time_delta_seq, f))
    dtb = np.ascontiguousarray(np.asarray(dtime_boundary, f))
    raw = np.ascontiguousarray(np.asarray(exp_raw, f))
    u = np.ascontiguousarray(np.asarray(unif_numbers, f))
    ev = np.asarray(event_seq)
    mu = np.asarray(mu, f)
    alpha = np.asarray(alpha, f)
    beta = np.asarray(beta, f)
    type_emb = np.asarray(type_emb, f)

    aemb_full = (alpha[None, :] * type_emb)[ev]            # [B,L,M]
    negbeta_bc = np.tile(-beta[None, :], (NP, 1)).astype(f)
    mu_bc = np.tile(mu[None, :], (NP, 1)).astype(f)
    tlin = np.linspace(0.0, 1.0, S0, dtype=f)
    tlin_bc = np.tile(tlin[None, :], (NP, 1)).astype(f)

    # Interpolation domain D per pair (float64 host estimate; only needs to
    # satisfy D >= xmax, which holds because bounds >= 1.5*tot(dt=0)).
    tot00 = np.log1p(np.exp((aemb_full + mu[None, None, :]).astype(np.float64))).sum(-1)
    rawmax = raw.max(-1).astype(np.float64)
    Ddom = rawmax / (1.5 * tot00)                          # [B,L]
    n = KC - 1
    jj = np.arange(KC)
    frac = (1.0 + np.cos(np.pi * jj / n)) / 2.0
    nodes_full = (Ddom[..., None] * frac[None, None, :]).astype(f)   # [B,L,KC]
    fourd_full = (4.0 / Ddom).astype(f)                    # [B,L]

    Wm = np.zeros((KC, KC))
    for k in range(KC):
        wrow = np.cos(np.pi * jj * k / n)
        wrow[0] *= 0.5
        wrow[-1] *= 0.5
        wrow *= 2.0 / n
        if k == 0 or k == n:
            wrow *= 0.5
        Wm[k] = wrow
    wfull_bc = np.tile(Wm.reshape(1, KC * KC).astype(f), (NP, 1))

    in_maps = []
    for c in range(N_CORES):
        bs = slice(c * BPC, (c + 1) * BPC)
        in_maps.append(dict(
            u=np.ascontiguousarray(u[bs].reshape(P, S, E)),
            raw=np.ascontiguousarray(raw[bs].reshape(P, E)),
            tds=np.ascontiguousarray(tds[bs].reshape(P, 1)),
            dtb=np.ascontiguousarray(dtb[bs].reshape(P, 1)),
            aemb=np.ascontiguousarray(aemb_full[bs].reshape(P, M)),
            nodes=np.ascontiguousarray(nodes_full[bs].reshape(P, KC)),
            fourd=np.ascontiguousarray(fourd_full[bs].reshape(P, 1)),
            negbeta=negbeta_bc,
            mu=mu_bc,
            tlin=tlin_bc,
            wfull=wfull_bc,
        ))
    return in_maps


def kernel(time_seq, time_delta_seq, event_seq, dtime_boundary, exp_raw,
           unif_numbers, mu, alpha, beta, type_emb, _trace=False):
    if "nc" not in _CACHE:
        _CACHE["nc"] = build_program()
    nc = _CACHE["nc"]

    in_maps = _prep_inputs(time_seq, time_delta_seq, event_seq, dtime_boundary,
                           exp_raw, unif_numbers, mu, alpha, beta, type_emb)

    out = run_bass_kernel_spmd(nc, in_maps, core_ids=list(range(N_CORES)),
                               trace=_trace)
    _CACHE["last_results"] = out

    res = np.concatenate([out.results[c]["res"].reshape(BPC, L, S)
                          for c in range(N_CORES)], axis=0)
    weights = np.full((B, L, S), np.float32(1.0 / S), np.float32)
    return res, weights



# revision 23
# speedup vs baseline: 1.1204x; 1.1204x over previous
"""Trainium2 Bass kernel for nn_EventSampler (thinning / rejection sampling).

kernel(**inputs) takes FULL unsharded inputs, shards batch across 8 cores
(2 batches = 256 (b,l) pairs per core), runs one SPMD Bass program, returns
the full output.

v3 structure (cost-model driven; per core):
  host prep: e-axis of (exp_raw, unif_numbers) sorted ascending by exp_raw
  per (b,l) pair, u transposed to [E, S, P] so the accept test runs in an
  e-on-partition layout.

  phase 1 (pair-layout, f32, same operation classes as the validated
  baseline): 25-point grid (20 bound-scan + 5 Chebyshev-Lobatto nodes) ->
  bounds -> b15/invb -> degree-4 polynomial tot(x) at the sorted draws ->
  q = tot*invb.  q is transposed to e-layout via PE transpose.

  phase 2 (e-layout, 16 pieces = 2 e-chunks x 8 s-slices):
    m = (u < q) on DVE (the only full-size f32 pass),
    first-accept extraction on PE: counts = (200*I - 200*strict_tri) @ m
    (+ cross-chunk -200*ones @ m0), IND = Act(Exp, counts - 200) in {0,1},
    fa = IND * raw_sorted (bf16, 2x DVE mode, split DVE/GpSimd),
    val = ones @ fa = raw* (or exact 0 if no accept) accumulated on PE.
  val rows return to pair-layout via PE transpose; tail (who = val>0,
  acc = val*invb, fallback max(last_raw*invb, dtb), min 1e5) is tiny.

Decision-critical arithmetic stays f32; bf16 only on the value path
(res tolerance 2e-2; host-sim on the real inputs shows max rel 3.9e-3).
"""

import os
import sys

import numpy as np

for _p in ("/opt/trn_rl_repo",):
    if _p not in sys.path and os.path.isdir(_p):
        sys.path.insert(0, _p)

import concourse.bacc as bacc
import concourse.tile as tile
import concourse.mybir as mybir
from concourse.bass_utils import run_bass_kernel_spmd

F32 = mybir.dt.float32
BF16 = mybir.dt.bfloat16

B, L, M = 16, 128, 32
S, E, S0 = 32, 256, 20
OVER = 1.5
KC = 5
G = S0 + KC
N_CORES = 8
BPC = B // N_CORES
P = BPC * L                     # pairs per core = 256
NP = 128
NCH = P // NP                   # pair chunks = 2
ECH = E // NP                   # e chunks = 2
RS = 4                          # s-rows per phase-2 piece
NSP = S // RS                   # s-pieces = 8
NPC = RS * P                    # free cols per piece = 1024
BIG = 200.0

# pk layout: aemb | tds | ddom | fourd | dtb | lastraw
WPK = M + 5
# cst (f32): mu | negbeta | linfrac | wfull(25) | t2m(25) | identity(128)
WCS = M + M + G + KC * KC + KC * KC + NP
# cstb (bf16): stc(128) | stall(128) | stone(1)
WCB = NP + NP + 1

FA_DVE_ROWS = 2                 # fa rows on DVE per piece (rest on GpSimd)

_CACHE = {}


def build_program():
    nc = bacc.Bacc("TRN2", target_bir_lowering=False, debug=False,
                   enable_asserts=False, num_devices=N_CORES)

    pk_d = nc.dram_tensor("pk", [P, WPK], F32, kind="ExternalInput")
    cst_d = nc.dram_tensor("cst", [NP, WCS], F32, kind="ExternalInput")
    cstb_d = nc.dram_tensor("cstb", [NP, WCB], BF16, kind="ExternalInput")
    rws_d = nc.dram_tensor("rws", [P, E], F32, kind="ExternalInput")
    rwt_d = nc.dram_tensor("rwt", [E, P], BF16, kind="ExternalInput")
    ut_d = nc.dram_tensor("ut", [E, S, P], F32, kind="ExternalInput")
    res_d = nc.dram_tensor("res", [P, S], F32, kind="ExternalOutput")
    DBG = os.environ.get("K_DBG") == "1"
    if DBG:
        mdbg_d = nc.dram_tensor("mdbg", [NP, RS, P], F32, kind="ExternalOutput")
        cdbg_d = nc.dram_tensor("cdbg", [NP, RS * P], F32, kind="ExternalOutput")
        idbg_d = nc.dram_tensor("idbg", [NP, RS, P], F32, kind="ExternalOutput")
        vdbg_d = nc.dram_tensor("vdbg", [1, RS * P], F32, kind="ExternalOutput")
        wdbg_d = nc.dram_tensor("wdbg", [S, P], F32, kind="ExternalOutput")

    alu = mybir.AluOpType
    act = mybir.ActivationFunctionType

    with tile.TileContext(nc) as tc:
        with (
            tc.tile_pool(name="io", bufs=1) as iop,
            tc.tile_pool(name="ubuf", bufs=1) as ubuf,
            tc.tile_pool(name="mbuf", bufs=1) as mbuf,
            tc.tile_pool(name="ph1", bufs=2) as ph1,
            tc.tile_pool(name="p2", bufs=3) as p2,
            tc.tile_pool(name="dbgp", bufs=1) as dbgp,
            tc.tile_pool(name="psA", bufs=2, space="PSUM") as psa,
            tc.tile_pool(name="psV", bufs=1, space="PSUM") as psv,
            tc.tile_pool(name="psQ", bufs=1, space="PSUM") as psq,
        ):
            # ---- DMAs: small packed inputs first, then the u stream ----
            pk_t = iop.tile([NP, NCH, WPK], F32, tag="pk")
            nc.sync.dma_start(out=pk_t[:],
                              in_=pk_d.ap().rearrange("(c p) w -> p c w", p=NP))
            cst_t = iop.tile([NP, WCS], F32, tag="cst")
            nc.sync.dma_start(out=cst_t[:], in_=cst_d.ap())
            cstb_t = iop.tile([NP, WCB], BF16, tag="cstb")
            nc.sync.dma_start(out=cstb_t[:], in_=cstb_d.ap())
            rws_t = iop.tile([NP, NCH, E], F32, tag="rws")
            nc.sync.dma_start(out=rws_t[:],
                              in_=rws_d.ap().rearrange("(c p) e -> p c e", p=NP))
            rwt_t = iop.tile([NP, ECH, P], BF16, tag="rwt")
            nc.sync.dma_start(out=rwt_t[:],
                              in_=rwt_d.ap().rearrange("(c a) p -> a c p", a=NP))

            ut = [ubuf.tile([NP, S, P], F32, tag=f"ut{a}", name=f"ut{a}")
                  for a in range(ECH)]
            for j in range(NSP):
                for a in range(ECH):
                    sl = slice(j * RS, (j + 1) * RS)
                    nc.sync.dma_start(
                        out=ut[a][:, sl, :],
                        in_=ut_d.ap().rearrange("(c a) s p -> a c s p", a=NP)
                            [:, a, sl, :])

            mu_t = cst_t[:, 0:M]
            negb_t = cst_t[:, M:2 * M]
            linfrac_t = cst_t[:, 2 * M:2 * M + G]
            o = 2 * M + G
            wfull_t = cst_t[:, o:o + KC * KC].rearrange("p (a b) -> p a b", a=KC)
            t2m_t = cst_t[:, o + KC * KC:o + 2 * KC * KC].rearrange(
                "p (a b) -> p a b", a=KC)
            ident_t = cst_t[:, o + 2 * KC * KC:]
            stc_t = cstb_t[:, 0:NP]
            stall_t = cstb_t[:, NP:2 * NP]
            stone_t = cstb_t[:, 2 * NP:2 * NP + 1]

            biasm = iop.tile([NP, 1], F32, tag="biasm")
            nc.gpsimd.memset(biasm[:], -BIG)

            negE = iop.tile([NP, G, M], F32, tag="negE")
            nc.vector.tensor_tensor(
                out=negE[:],
                in0=linfrac_t.unsqueeze(2).to_broadcast((NP, G, M)),
                in1=negb_t.unsqueeze(1).to_broadcast((NP, G, M)), op=alu.mult)

            # qT[a]: [e-part, pair(c major)] built by PE transpose per chunk
            qT = psq.tile([NP, ECH, NP * NCH], F32, tag="qT")

            ch = [dict() for _ in range(NCH)]
            for c in range(NCH):
                d = ch[c]
                aemb = pk_t[:, c, 0:M]
                tds = pk_t[:, c, M:M + 1]
                ddom = pk_t[:, c, M + 1:M + 2]
                fourd = pk_t[:, c, M + 2:M + 3]
                d["dtb"] = pk_t[:, c, M + 3:M + 4]
                d["lastraw"] = pk_t[:, c, M + 4:M + 5]
                raw = rws_t[:, c, :]

                eng = nc.vector if c == 0 else nc.gpsimd
                dG = ph1.tile([NP, G, M], F32, tag="gA", name=f"dG{c}")
                nc.scalar.activation(dG[:, 0:S0, :], negE[:, 0:S0, :], act.Exp,
                                     scale=tds)
                nc.scalar.activation(dG[:, S0:G, :], negE[:, S0:G, :], act.Exp,
                                     scale=ddom)
                gG = ph1.tile([NP, G, M], F32, tag="gB", name=f"gG{c}")
                eng.tensor_tensor(out=gG[:], in0=dG[:],
                                  in1=aemb.unsqueeze(1).to_broadcast((NP, G, M)),
                                  op=alu.mult)
                sG = ph1.tile([NP, G, M], F32, tag="gA", name=f"sG{c}")
                eng.tensor_tensor(out=sG[:], in0=gG[:],
                                  in1=mu_t.unsqueeze(1).to_broadcast((NP, G, M)),
                                  op=alu.add)
                eG = ph1.tile([NP, G, M], F32, tag="gB", name=f"eG{c}")
                nc.scalar.activation(eG[:], sG[:], act.Exp)
                spG = ph1.tile([NP, G, M], F32, tag="gA", name=f"spG{c}")
                nc.scalar.activation(spG[:], eG[:], act.Ln, bias=1.0)
                vals = ph1.tile([NP, G], F32, tag="vals", name=f"vals{c}")
                nc.vector.reduce_sum(out=vals[:], in_=spG[:],
                                     axis=mybir.AxisListType.X)

                bmax = ph1.tile([NP, 1], F32, tag="bmax", name=f"bmax{c}")
                nc.vector.reduce_max(out=bmax[:], in_=vals[:, 0:S0],
                                     axis=mybir.AxisListType.X)
                b15 = ph1.tile([NP, 1], F32, tag="b15", name=f"b15{c}")
                nc.vector.tensor_scalar(out=b15[:], in0=bmax[:],
                                        scalar1=float(OVER), scalar2=None,
                                        op0=alu.mult)
                invb = ph1.tile([NP, 1], F32, tag="invb", name=f"invb{c}")
                nc.vector.reciprocal(invb[:], b15[:])
                svc2 = ph1.tile([NP, 1], F32, tag="svc2", name=f"svc2{c}")
                nc.vector.tensor_scalar(out=svc2[:], in0=invb[:], scalar1=fourd,
                                        scalar2=None, op0=alu.mult)
                w2 = ph1.tile([NP, E], F32, tag="w2", name=f"w2{c}")
                nc.vector.tensor_scalar(out=w2[:], in0=raw, scalar1=svc2[:],
                                        scalar2=-2.0, op0=alu.mult, op1=alu.add)

                cw = ph1.tile([NP, KC, KC], F32, tag="cw", name=f"cw{c}")
                nc.vector.tensor_tensor(
                    out=cw[:],
                    in0=vals[:, S0:G].unsqueeze(1).to_broadcast((NP, KC, KC)),
                    in1=wfull_t, op=alu.mult)
                cc = ph1.tile([NP, KC], F32, tag="cc", name=f"cc{c}")
                nc.vector.reduce_sum(out=cc[:], in_=cw[:],
                                     axis=mybir.AxisListType.X)
                cw2 = ph1.tile([NP, KC, KC], F32, tag="cw2", name=f"cw2{c}")
                nc.vector.tensor_tensor(
                    out=cw2[:],
                    in0=cc[:].unsqueeze(1).to_broadcast((NP, KC, KC)),
                    in1=t2m_t, op=alu.mult)
                am = ph1.tile([NP, KC], F32, tag="am", name=f"am{c}")
                nc.vector.reduce_sum(out=am[:], in_=cw2[:],
                                     axis=mybir.AxisListType.X)

                x2 = ph1.tile([NP, E], F32, tag="x2", name=f"x2{c}")
                nc.vector.tensor_tensor(out=x2[:], in0=w2[:], in1=w2[:],
                                        op=alu.mult)
                u1 = ph1.tile([NP, E], F32, tag="u1", name=f"u1{c}")
                nc.vector.tensor_scalar(out=u1[:], in0=x2[:],
                                        scalar1=am[:, 4:5], scalar2=am[:, 2:3],
                                        op0=alu.mult, op1=alu.add)
                u2 = ph1.tile([NP, E], F32, tag="u2", name=f"u2{c}")
                nc.vector.tensor_tensor(out=u2[:], in0=u1[:], in1=x2[:],
                                        op=alu.mult)
                v1 = ph1.tile([NP, E], F32, tag="v1", name=f"v1{c}")
                nc.vector.tensor_scalar(out=v1[:], in0=x2[:],
                                        scalar1=am[:, 3:4], scalar2=am[:, 1:2],
                                        op0=alu.mult, op1=alu.add)
                v2 = ph1.tile([NP, E], F32, tag="v2", name=f"v2{c}")
                nc.vector.tensor_tensor(out=v2[:], in0=v1[:], in1=w2[:],
                                        op=alu.mult)
                tot = ph1.tile([NP, E], F32, tag="tot", name=f"tot{c}")
                nc.vector.scalar_tensor_tensor(out=tot[:], in0=u2[:],
                                               scalar=am[:, 0:1], in1=v2[:],
                                               op0=alu.add, op1=alu.add)
                q = ph1.tile([NP, E], F32, tag="q", name=f"q{c}")
                nc.vector.tensor_scalar(out=q[:], in0=tot[:], scalar1=invb[:],
                                        scalar2=None, op0=alu.mult)
                # q [pair, e] -> qT [e, pair] (PE transpose per e-chunk block)
                for a in range(ECH):
                    nc.tensor.transpose(
                        qT[:, a, c * NP:(c + 1) * NP],
                        q[:, a * NP:(a + 1) * NP], ident_t)
                d.update(invb=invb)

            # ---- phase 2: 16 pieces ----
            vst = iop.tile([S, P], F32, tag="vst")
            m_t = [mbuf.tile([NP, S, P], BF16, tag=f"m{a}", name=f"m{a}")
                   for a in range(ECH)]
            for j in range(NSP):
                sl = slice(j * RS, (j + 1) * RS)
                for a in range(ECH):
                    nc.vector.tensor_tensor(
                        out=m_t[a][:, sl, :], in0=ut[a][:, sl, :],
                        in1=qT[:, a, :].unsqueeze(1).to_broadcast((NP, RS, P)),
                        op=alu.is_lt)
                    mv = m_t[a][:, sl, :].rearrange("a s p -> a (s p)")
                    cnt = psa.tile([NP, NPC], F32, tag="cnt", name=f"cnt{a}_{j}")
                    HH = NPC // 2
                    for h in range(2):
                        hs = slice(h * HH, (h + 1) * HH)
                        if a == 0:
                            nc.tensor.matmul(cnt[:, hs], stc_t, mv[:, hs],
                                             start=True, stop=True)
                        else:
                            nc.tensor.matmul(cnt[:, hs], stc_t, mv[:, hs],
                                             start=True, stop=False)
                            nc.tensor.matmul(
                                cnt[:, hs], stall_t,
                                m_t[0][:, sl, :].rearrange("a s p -> a (s p)")[:, hs],
                                start=False, stop=True)
                    ind = p2.tile([NP, RS, P], BF16, tag="ind", name=f"ind{a}_{j}")
                    nc.scalar.activation(ind[:].rearrange("a s p -> a (s p)"),
                                         cnt[:], act.Exp, bias=biasm[:])
                    rbc = rwt_t[:, a, :].unsqueeze(1)
                    fd = FA_DVE_ROWS
                    fa_d = p2.tile([NP, fd, P], BF16, tag="fad", name=f"fad{a}_{j}")
                    nc.vector.tensor_tensor(
                        out=fa_d[:], in0=ind[:, 0:fd, :],
                        in1=rbc.to_broadcast((NP, fd, P)), op=alu.mult)
                    fa_p = p2.tile([NP, RS - fd, P], BF16, tag="fap",
                                   name=f"fap{a}_{j}")
                    nc.gpsimd.tensor_tensor(
                        out=fa_p[:], in0=ind[:, fd:RS, :],
                        in1=rbc.to_broadcast((NP, RS - fd, P)), op=alu.mult)
                    if a == 0:
                        pv = psv.tile([1, NPC], F32, tag="pv", name=f"pv{j}")
                        pv_hold = pv
                    else:
                        pv = pv_hold
                    HHV = NPC // 2
                    nc.tensor.matmul(pv[:, 0:HHV], stone_t,
                                     fa_d[:].rearrange("a s p -> a (s p)"),
                                     start=(a == 0), stop=(a == 1))
                    nc.tensor.matmul(pv[:, HHV:NPC], stone_t,
                                     fa_p[:].rearrange("a s p -> a (s p)"),
                                     start=(a == 0), stop=(a == 1))
                    if DBG and j == 1 and a == 0:
                        t1 = dbgp.tile([NP, RS, P], F32, tag="dbg1")
                        nc.vector.tensor_copy(t1[:], m_t[a][:, sl, :])
                        nc.sync.dma_start(out=mdbg_d.ap(), in_=t1[:])
                        t2 = dbgp.tile([NP, RS * P], F32, tag="dbg2")
                        nc.vector.tensor_copy(t2[:], cnt[:])
                        nc.sync.dma_start(out=cdbg_d.ap(), in_=t2[:])
                        t3 = dbgp.tile([NP, RS, P], F32, tag="dbg3")
                        nc.vector.tensor_copy(t3[:], ind[:])
                        nc.sync.dma_start(out=idbg_d.ap(), in_=t3[:])
                    if DBG and j == 1 and a == 1:
                        t4 = dbgp.tile([1, RS * P], F32, tag="dbg4")
                        nc.vector.tensor_copy(t4[:], pv[:])
                        nc.sync.dma_start(out=vdbg_d.ap(), in_=t4[:])
                    if a == 1:
                        svj = p2.tile([1, NPC], F32, tag=f"sv{j % 2}",
                                      name=f"sv{j}")
                        if j % 2 == 0:
                            nc.scalar.activation(svj[:], pv[:], act.Copy)
                        else:
                            nc.vector.tensor_copy(svj[:], pv[:])
                        nc.sync.dma_start(
                            out=vst[j * RS:(j + 1) * RS, :],
                            in_=svj[:].rearrange("o (s p) -> o s p", s=RS))

            if DBG:
                nc.sync.dma_start(out=wdbg_d.ap(), in_=vst[:])
            # PE-transpose the collected val rows back to pair-layout
            valT = psq.tile([NP, NCH, S], F32, tag="valT")
            for c in range(NCH):
                nc.tensor.transpose(valT[:, c, :], vst[:, c * NP:(c + 1) * NP],
                                    ident_t[0:S, 0:S])

            for c in range(NCH):
                d = ch[c]
                invb = d["invb"]
                val = valT[:, c, :]
                who = ph1.tile([NP, S], mybir.dt.int32, tag="who", name=f"who{c}")
                nc.vector.tensor_scalar(out=who[:], in0=val, scalar1=0.0,
                                        scalar2=None, op0=alu.is_gt)
                acc = ph1.tile([NP, S], F32, tag="acc", name=f"acc{c}")
                nc.vector.tensor_scalar(out=acc[:], in0=val, scalar1=invb[:],
                                        scalar2=None, op0=alu.mult)
                lastx = ph1.tile([NP, 1], F32, tag="lastx", name=f"lastx{c}")
                nc.vector.tensor_scalar(out=lastx[:], in0=d["lastraw"],
                                        scalar1=invb[:], scalar2=None,
                                        op0=alu.mult)
                fb = ph1.tile([NP, 1], F32, tag="fb", name=f"fb{c}")
                nc.vector.tensor_tensor(out=fb[:], in0=lastx[:], in1=d["dtb"],
                                        op=alu.max)
                res_t = ph1.tile([NP, S], F32, tag="res", name=f"res{c}")
                nc.scalar.activation(res_t[:], fb[:].to_broadcast((NP, S)),
                                     act.Copy)
                nc.vector.copy_predicated(res_t[:], who[:], acc[:])
                res2_t = ph1.tile([NP, S], F32, tag="res2", name=f"res2{c}")
                nc.vector.tensor_scalar(out=res2_t[:], in0=res_t[:],
                                        scalar1=1e5, scalar2=None, op0=alu.min)
                nc.sync.dma_start(out=res_d.ap()[c * NP:(c + 1) * NP],
                                  in_=res2_t[:])

    nc.finalize()
    return nc


def _prep_inputs(time_seq, time_delta_seq, event_seq, dtime_boundary, exp_raw,
                 unif_numbers, mu, alpha, beta, type_emb):
    f = np.float32
    tds = np.asarray(time_delta_seq, f).reshape(B * L)
    dtb = np.asarray(dtime_boundary, f).reshape(B * L)
    raw0 = np.asarray(exp_raw, f).reshape(B * L, E)
    u = np.asarray(unif_numbers, f).reshape(B * L, S, E)
    ev = np.asarray(event_seq)
    mu = np.asarray(mu, f)
    alpha = np.asarray(alpha, f)
    beta = np.asarray(beta, f)
    type_emb = np.asarray(type_emb, f)

    aemb = (alpha[None, :] * type_emb)[ev].reshape(B * L, M).astype(f)

    order = np.argsort(raw0, axis=-1, kind="stable")
    raws = np.take_along_axis(raw0, order, axis=-1).astype(f)
    us = np.take_along_axis(u, order[:, None, :], axis=-1).astype(f)

    tot00 = np.log1p(np.exp((aemb + mu[None, :]).astype(np.float64))).sum(-1)
    rawmax = raw0.max(-1).astype(np.float64)
    Ddom = rawmax / (1.5 * tot00)
    fourd = (4.0 / Ddom).astype(f)
    ddom = Ddom.astype(f)

    jj = np.arange(KC)
    n = KC - 1
    frac = (1.0 + np.cos(np.pi * jj / n)) / 2.0
    linfrac = np.concatenate([np.linspace(0.0, 1.0, S0), frac]).astype(f)

    Wm = np.zeros((KC, KC))
    for k in range(KC):
        wrow = np.cos(np.pi * jj * k / n)
        wrow[0] *= 0.5
        wrow[-1] *= 0.5
        wrow *= 2.0 / n
        if k == 0 or k == n:
            wrow *= 0.5
        Wm[k] = wrow
    # tot = sum_k cc_k T_k(w2/2); T_k(w2/2) as powers of w2 (cols = power)
    t2m = np.zeros((KC, KC))
    t2m[0, 0] = 1.0
    t2m[1, 1] = 0.5
    t2m[2, 0], t2m[2, 2] = -1.0, 0.5
    t2m[3, 1], t2m[3, 3] = -1.5, 0.5
    t2m[4, 0], t2m[4, 2], t2m[4, 4] = 1.0, -2.0, 0.5
    # cw2 uses cc broadcast over rows a: am_j = sum_b cc_b * t2m[b, j]
    t2m_packed = t2m.T.reshape(1, KC * KC)  # [a=j(power), b=k(cheb)] row-major

    def bf16_bytes(x):
        x = np.ascontiguousarray(np.asarray(x, np.float32))
        u32 = x.view(np.uint32)
        r = ((u32 + 0x7FFF + ((u32 >> 16) & 1)) >> 16).astype(np.uint16)
        return r

    cst = np.concatenate([
        np.tile(mu[None, :], (NP, 1)),
        np.tile(-beta[None, :], (NP, 1)),
        np.tile(linfrac[None, :], (NP, 1)),
        np.tile(Wm.reshape(1, KC * KC).astype(f), (NP, 1)),
        np.tile(t2m_packed.astype(f), (NP, 1)),
        np.eye(NP, dtype=f),
    ], axis=1).astype(f)

    stc = np.zeros((NP, NP), f)
    for k in range(NP):
        stc[k, k] = BIG
        stc[:k, k] = -BIG
    stall = np.full((NP, NP), -BIG, f)
    stone = np.ones((NP, 1), f)
    cstb = np.concatenate([bf16_bytes(stc), bf16_bytes(stall),
                           bf16_bytes(stone)], axis=1)

    pk = np.concatenate([
        aemb, tds[:, None], ddom[:, None], fourd[:, None], dtb[:, None],
        raw0[:, E - 1:E],
    ], axis=1).astype(f)

    in_maps = []
    for core in range(N_CORES):
        rs = slice(core * P, (core + 1) * P)
        uT = np.ascontiguousarray(us[rs].transpose(2, 1, 0))   # [E, S, P]
        rwt = bf16_bytes(raws[rs].T)                           # [E, P] bf16
        in_maps.append(dict(
            pk=np.ascontiguousarray(pk[rs]),
            cst=cst,
            cstb=cstb,
            rws=np.ascontiguousarray(raws[rs]),
            rwt=np.ascontiguousarray(rwt),
            ut=uT,
        ))
    return in_maps


def kernel(time_seq, time_delta_seq, event_seq, dtime_boundary, exp_raw,
           unif_numbers, mu, alpha, beta, type_emb, _trace=False):
    if "nc" not in _CACHE:
        _CACHE["nc"] = build_program()
    nc = _CACHE["nc"]

    in_maps = _prep_inputs(time_seq, time_delta_seq, event_seq, dtime_boundary,
                           exp_raw, unif_numbers, mu, alpha, beta, type_emb)

    out = run_bass_kernel_spmd(nc, in_maps, core_ids=list(range(N_CORES)),
                               trace=_trace)
    _CACHE["last_results"] = out

    res = np.concatenate([out.results[c]["res"].reshape(BPC, L, S)
                          for c in range(N_CORES)], axis=0)
    weights = np.full((B, L, S), np.float32(1.0 / S), np.float32)
    return res, weights
